# revision 29
# baseline (speedup 1.0000x reference)
"""BrushStroke splat kernel for 8 trn2 NeuronCores.

out[b,c,y,x] = mean_n sum_{p,q} Fy[b,n,y,p] Fx[b,n,x,q] patches[b,n,c,p,q]
with Fx/Fy separable Gaussian filter banks (sigma=0.1) normalized over a
padded spatial axis.

Key insight: sigma=0.1 makes every normalized filter column an EXACT
2-tap tent:  Fx[n,x,q] = s_n * relu(1 - |x - u*|),  u* = floor(ux)+beta,
s = S/(S+EPS).  So the filter tiles are built on-chip from per-stroke
scalars with two ScalarE activations (Fx) / four DVE ops (Fy) per group
of 4 strokes -- no DMA gathers (the old Toeplitz gathers saturated the 4
HW DMA engines at ~160us).

Per core (2 batches x 64 strokes), one software-pipelined loop over 32
(batch, group) steps:
  A: one-hot f32r matmul broadcasts per-stroke scalars [6] to the
     (j, q') 128-partition layout; tent biases via tiny DVE subs.
  B: tents: fxn = Relu(-s*Abs(x - u*) + s) on ScalarE (f16),
     fyn likewise on DVE with 1/N folded in.
  C: MM1 (f16): block-diag patches lhsT x fxn -> t (PSUM), cast-copied
     to bf16 (c0+c1 fused in one 512-wide tile).
  D: MM2 (f16): 4 accumulating matmuls/group into 4 PSUM banks
     (2 y-tiles x {c01 fused, c2}), chains interleaved with stage C.
Batch-parallel across cores; no collectives.
"""
import sys, types
import numpy as np

IMAGE = 256
EPS = 1e-7
B, N, C, PH, PW = 16, 64, 3, 32, 32
NCORES = 8
BLOC = B // NCORES          # 2 batches per core
NG = N // 4                 # 16 groups of 4 strokes
NSTEP = BLOC * NG           # 32 pipeline steps


def _install_patches():
    if 'antenv.axon_hooks' not in sys.modules:
        mod = types.ModuleType('antenv.axon_hooks')
        mod._hook = None
        mod.set_axon_ntff_profile_hook = lambda h: setattr(mod, '_hook', h)
        mod.get_axon_ntff_profile_hook = lambda: mod._hook
        sys.modules['antenv.axon_hooks'] = mod
        try:
            from trn_agent_boot.trn_boot import _ntff_profile_via_ctypes
            hook = _ntff_profile_via_ctypes('/opt/axon/libaxon_pjrt.so')
            if hook is not None:
                mod.set_axon_ntff_profile_hook(hook)
        except Exception:
            pass

    import concourse.tile as tile
    import concourse.bass_utils as bass_utils
    from concourse.vector_clock import ScopedClock

    bass_utils.upload_artifacts = lambda tmpdir: 'local://' + tmpdir

    if getattr(tile.TileContext._drain_and_barrier, '_patched', False):
        return

    def _drain_and_barrier(self, tick_clock, wait_clock):
        nc = self.nc
        drain_inst = nc.sync.drain()
        wait_clock.add_sem_waits(
            drain_inst.ins, ScopedClock({None: tick_clock.global_clock}))
        si = drain_inst.ins.sync_info
        waits = list(si.on_wait or [])
        si.on_wait = []
        for w in waits:
            nop = nc.sync.nop()
            nop.ins.sync_info = type(si)(on_wait=[w], on_update=[])
        nc.all_engine_barrier()
        popped = nc._tile_sem_poison_stack.pop()
        assert popped is self._sem_poison
        nc.clear_and_free_semaphores(list(self.sems.allocated().values()))
        nc.all_engine_barrier()

    _drain_and_barrier._patched = True
    tile.TileContext._drain_and_barrier = _drain_and_barrier


def _split_multi_waits(nc):
    """This walrus accepts at most one sync wait per instruction; hoist
    extras onto same-engine NoOps inserted just before."""
    import bass_rust
    n_new = [0]

    def fresh_nop(engine, wait, si_type):
        n_new[0] += 1
        nop = bass_rust.InstNoOp(name=f'I-waitsplit-{n_new[0]}', ins=[], outs=[])
        nop.engine = engine
        nop.sync_info = si_type(on_wait=[wait], on_update=[])
        return nop

    for fn in nc.m.functions:
        for blk in fn.blocks:
            insts = blk.instructions
            i = 0
            while i < len(insts):
                inst = insts[i]
                si = inst.sync_info
                if si is not None and si.on_wait and len(si.on_wait) > 1:
                    waits = list(si.on_wait)
                    si.on_wait = [waits[-1]]
                    for k, w in enumerate(waits[:-1]):
                        insts.insert(i + k, fresh_nop(inst.engine, w, type(si)))
                    i += len(waits) - 1
                i += 1


_PROGRAM = None


def _build_program():
    global _PROGRAM
    if _PROGRAM is not None:
        return _PROGRAM
    _install_patches()
    import concourse.bass as bass
    import concourse.tile as tile
    from concourse import mybir

    f32 = mybir.dt.float32
    f32r = mybir.dt.float32r
    f16 = mybir.dt.float16
    AF = mybir.ActivationFunctionType
    AX = mybir.AxisListType
    OP = mybir.AluOpType

    nc = bass.Bass('TRN2', target_bir_lowering=False, debug=False,
                   num_devices=NCORES)
    g_in = nc.declare_dram_parameter('g_in', [4, N], f32, isOutput=False)
    pt_in = nc.declare_dram_parameter('pt_in', [BLOC, 128, NG * C * PH], f16,
                                      isOutput=False)
    id4 = nc.declare_dram_parameter('id4', [4, 4], f32, isOutput=False)
    bmat = nc.declare_dram_parameter('bmat', [128, NG * 128], f32r,
                                     isOutput=False)
    qio_in = nc.declare_dram_parameter('qio_in', [128, 1], f32, isOutput=False)
    it_in = nc.declare_dram_parameter('it_in', [128, IMAGE], f16,
                                      isOutput=False)
    y_out = nc.declare_dram_parameter('y_out', [BLOC, C, IMAGE, IMAGE], f32,
                                      isOutput=True)

    with tile.TileContext(nc) as tc:
        with tc.tile_pool(name='main', bufs=1) as gp, \
             tc.tile_pool(name='ps', bufs=1, space='PSUM') as pp:
            # ---- constant / input loads ----
            bc = gp.tile([4, N], f32)
            nc.sync.dma_start(bc[:], g_in[:])
            idt = gp.tile([4, 4], f32)
            nc.sync.dma_start(idt[:], id4[:])
            bmt = gp.tile([128, NG * 128], f32r)
            nc.sync.dma_start(bmt[:], bmat[:])
            qio = gp.tile([128, 1], f32)
            nc.sync.dma_start(qio[:], qio_in[:])
            it_b = gp.tile([128, IMAGE], f16)
            nc.sync.dma_start(it_b[:], it_in[:])
            ptc = []
            for b in range(BLOC):
                t = gp.tile([128, NG * C * PH], f16, name=f'ptc{b}')
                nc.scalar.dma_start(t[:], pt_in[b])
                ptc.append(t)

            # ---- ps_all: block-diagonal patch weights, both batches ----
            from bass_rust import AP
            ps_all = []
            for b in range(BLOC):
                pa = gp.tile([128, NG * C * 128], f16, name=f'psall{b}')
                nc.gpsimd.memset(pa.bitcast(f32)[:], 0.0)
                ps_all.append(pa)
            for b in range(BLOC):
                for j in range(4):
                    dst0 = ps_all[b][32 * j:32 * j + 1, 32 * j:32 * j + 1]
                    dst = AP(ps_all[b].tensor, dst0.offset,
                             [[128 * C * NG, 32], [128 * C, NG],
                              [128, C], [1, PH]])
                    src0 = ptc[b][32 * j:32 * j + 1, 0:1]
                    srcap = AP(ptc[b].tensor, src0.offset,
                               [[NG * C * PH, 32], [C * PH, NG],
                                [PH, C], [1, PH]])
                    eng = nc.vector if j % 2 == 0 else nc.gpsimd
                    eng.tensor_copy(dst, srcap)

            # ---- brush normalization -> per-stroke scalar table ----
            mn = gp.tile([4, 1], f32)
            mx = gp.tile([4, 1], f32)
            nc.vector.tensor_reduce(mn[:], bc[:], axis=AX.X, op=OP.min)
            nc.vector.reduce_max(mx[:], bc[:], axis=AX.X)
            rng = gp.tile([4, 1], f32)
            nc.vector.tensor_sub(rng[:], mx[:], mn[:])
            nc.vector.tensor_scalar_add(rng[:], rng[:], EPS)
            inv = gp.tile([4, 1], f32)
            nc.vector.reciprocal(inv[:], rng[:])
            nc.vector.tensor_scalar_mul(inv[:], inv[:], float(IMAGE))
            gn = gp.tile([4, N], f32)
            nc.vector.tensor_scalar_sub(gn[:], bc[:], mn[:])
            nc.vector.tensor_scalar_mul(gn[:], gn[:], inv[:])

            # PSUM is 8 banks x 2KB/partition; pack manually (exactly 8):
            #   accA0, accA1            2 banks (MM2 c0+c1 accumulators)
            #   accB0, accB1            2 banks (MM2 c2 accumulators)
            #   p01 (bufs=2)            2 banks (MM1 c0+c1)
            #   p2rot (manual 2-slot)   1 bank  (MM1 c2)
            #   pgp_rot (manual 3-slot) 1 bank  (scalar broadcast, + tp_ps)
            pgp_rot = pp.tile([128, 18], f32, name='pgp_rot')
            p2rot = pp.tile([128, 512], f32, name='p2rot')

            # transpose [4,N] -> [N,4]; stack both batches along partitions
            tp_ps = pgp_rot[0:N, 0:4]
            nc.tensor.transpose(tp_ps, gn[:], idt[:])
            tp = gp.tile([128, 4], f32)
            nc.scalar.copy(tp[0:N, :], tp_ps)
            nc.vector.tensor_copy(tp[N:128, :], tp_ps)

            # u0 (shifted +16 so mod(.,1) is a clean fractional part):
            # col0 = gx + 0.5, col1 = gy + 0.6
            u0 = gp.tile([128, 2], f32)
            nc.vector.tensor_scalar_add(u0[0:N, 0:1], tp[0:N, 0:1], 0.5)
            nc.vector.tensor_scalar_add(u0[N:128, 0:1], tp[N:128, 2:3], 0.5)
            nc.vector.tensor_scalar_add(u0[0:N, 1:2], tp[0:N, 1:2], 0.6)
            nc.vector.tensor_scalar_add(u0[N:128, 1:2], tp[N:128, 3:4], 0.6)

            # floor via round-to-nearest (2^23 trick) then fix-up:
            # fiv = round(u0) - (round(u0) > u0)
            tr = gp.tile([128, 2], f32)
            nc.vector.tensor_scalar(tr[:], u0[:], float(1 << 23),
                                    float(1 << 23), op0=OP.add,
                                    op1=OP.subtract)
            gt = gp.tile([128, 2], f32)
            nc.vector.tensor_tensor(gt[:], tr[:], u0[:], op=OP.is_gt)
            fiv = gp.tile([128, 2], f32)
            nc.vector.tensor_sub(fiv[:], tr[:], gt[:])
            fr = gp.tile([128, 2], f32)
            nc.vector.tensor_sub(fr[:], u0[:], fiv[:])

            sq1 = gp.tile([128, 2], f32)
            nc.scalar.activation(sq1[:], fr[:], AF.Square, bias=0.0, scale=1.0)
            a_t = gp.tile([128, 2], f32)
            nc.scalar.activation(a_t[:], sq1[:], AF.Exp, bias=0.0, scale=-50.0)
            sq2 = gp.tile([128, 2], f32)
            nc.scalar.activation(sq2[:], fr[:], AF.Square, bias=1.0,
                                 scale=-1.0)
            b_t = gp.tile([128, 2], f32)
            nc.scalar.activation(b_t[:], sq2[:], AF.Exp, bias=0.0, scale=-50.0)

            S = gp.tile([128, 2], f32)
            nc.vector.tensor_add(S[:], a_t[:], b_t[:])
            Sp = gp.tile([128, 2], f32)
            nc.vector.tensor_scalar_add(Sp[:], S[:], EPS)
            rS = gp.tile([128, 2], f32)
            nc.vector.reciprocal(rS[:], S[:])
            rSp = gp.tile([128, 2], f32)
            nc.vector.reciprocal(rSp[:], Sp[:])
            s_v = gp.tile([128, 2], f32)
            nc.vector.tensor_mul(s_v[:], S[:], rSp[:])
            bh = gp.tile([128, 2], f32)
            nc.vector.tensor_mul(bh[:], b_t[:], rS[:])

            # S_tab cols: 0 Ux=fi_x+bh_x, 1 -s_x, 2 +s_x,
            #             3 Uy=fi_y+bh_y, 4 -s_y/64, 5 +s_y/64
            S_tab = gp.tile([128, 6], f32r)
            nc.vector.tensor_add(S_tab[:, 0:1], fiv[:, 0:1], bh[:, 0:1])
            nc.vector.tensor_scalar_mul(S_tab[:, 1:2], s_v[:, 0:1], -1.0)
            nc.vector.tensor_copy(S_tab[:, 2:3], s_v[:, 0:1])
            nc.vector.tensor_add(S_tab[:, 3:4], fiv[:, 1:2], bh[:, 1:2])
            nc.vector.tensor_scalar_mul(S_tab[:, 4:5], s_v[:, 1:2], -1.0 / N)
            nc.vector.tensor_scalar_mul(S_tab[:, 5:6], s_v[:, 1:2], 1.0 / N)

            # ---- PSUM accumulators (shared across batches) ----
            accA = [pp.tile([128, 512], f32, name=f'accA{yt}')
                    for yt in range(2)]
            accB = [pp.tile([128, 256], f32, name=f'accB{yt}')[:]
                    for yt in range(2)]

            # ---- pipelined main loop ----
            pg_t, bias_t, fxn_t, fyn_t, t01_t, t2_t = {}, {}, {}, {}, {}, {}

            def stageA(k):
                b, g = k // NG, k % NG
                c0 = 6 * (k % 3)
                pgp = pgp_rot[:, c0:c0 + 6]
                s0 = N * b
                nc.tensor.matmul(pgp,
                                 bmt[s0:s0 + N, 128 * g:128 * g + 128],
                                 S_tab[s0:s0 + N, :],
                                 start=True, stop=True)
                pg = gp.tile([128, 6], f32, name='pg', tag='pg', bufs=3)
                nc.scalar.copy(pg[:], pgp)
                bias = gp.tile([128, 2], f32, name='bias', tag='bias', bufs=3)
                nc.vector.tensor_sub(bias[:, 0:1], qio[:], pg[:, 0:1])
                nc.vector.tensor_sub(bias[:, 1:2], qio[:], pg[:, 3:4])
                pg_t[k], bias_t[k] = pg, bias

            def stageB(k):
                pg, bias = pg_t[k], bias_t[k]
                abx = gp.tile([128, IMAGE], f16, name='abx', tag='abx',
                              bufs=2)
                nc.scalar.activation(abx[:], it_b[:], AF.Abs,
                                     bias=bias[:, 0:1], scale=1.0)
                fxn = gp.tile([128, IMAGE], f16, name='fxn', tag='fxn',
                              bufs=2)
                nc.scalar.activation(fxn[:], abx[:], AF.Relu,
                                     bias=pg[:, 2:3], scale=pg[:, 1:2])
                fxn_t[k] = fxn

                dv = gp.tile([128, IMAGE], f16, name='dv', tag='dv', bufs=2)
                nc.vector.tensor_scalar(dv[:], it_b[:], bias[:, 1:2], None,
                                        op0=OP.add)
                ab = gp.tile([128, IMAGE], f16, name='ab', tag='ab', bufs=2)
                nc.vector.scalar_tensor_tensor(ab[:], dv[:], -1.0, dv[:],
                                               op0=OP.mult, op1=OP.max)
                wv = gp.tile([128, IMAGE], f16, name='wv', tag='wv', bufs=2)
                nc.vector.tensor_scalar(wv[:], ab[:], pg[:, 4:5], pg[:, 5:6],
                                        op0=OP.mult, op1=OP.add)
                fyn = gp.tile([128, IMAGE], f16, name='fyn', tag='fyn',
                              bufs=3)
                nc.vector.tensor_scalar(fyn[:], wv[:], 0.0, None, op0=OP.max)
                fyn_t[k] = fyn

            def stageC(k):
                b, g = k // NG, k % NG
                fxn = fxn_t.pop(k)
                p01 = pp.tile([128, 512], f32, name='p01', tag='p01', bufs=2)
                pc0 = 256 * (k % 2)
                p2 = p2rot[:, pc0:pc0 + 256]
                base = 384 * g
                nc.tensor.matmul(p01[:, 0:256],
                                 ps_all[b][:, base: base + 128],
                                 fxn[:], start=True, stop=True)
                nc.tensor.matmul(p01[:, 256:512],
                                 ps_all[b][:, base + 128: base + 256],
                                 fxn[:], start=True, stop=True)
                nc.tensor.matmul(p2,
                                 ps_all[b][:, base + 256: base + 384],
                                 fxn[:], start=True, stop=True)
                t01 = gp.tile([128, 512], f16, name='t01', tag='t01', bufs=3)
                nc.vector.tensor_copy(t01[:], p01[:])
                t2 = gp.tile([128, 256], f16, name='t2', tag='t2', bufs=3)
                nc.scalar.copy(t2[:], p2)
                t01_t[k], t2_t[k] = t01, t2

            def stageD(k):
                b, g = k // NG, k % NG
                fyn, t01, t2 = fyn_t.pop(k), t01_t.pop(k), t2_t.pop(k)
                st, sp = (g == 0), (g == NG - 1)
                for yt in range(2):
                    nc.tensor.matmul(accA[yt][:],
                                     fyn[:, 128 * yt:128 * yt + 128],
                                     t01[:], start=st, stop=sp)
                    nc.tensor.matmul(accB[yt],
                                     fyn[:, 128 * yt:128 * yt + 128],
                                     t2[:], start=st, stop=sp)

            def drain(b):
                for yt in range(2):
                    obA = gp.tile([128, 512], f32, name=f'obA{yt}',
                                  tag=f'obA{yt}', bufs=2)
                    if yt == 0:
                        nc.vector.tensor_copy(obA[:], accA[yt][:])
                    else:
                        nc.scalar.copy(obA[:], accA[yt][:])
                    obB = gp.tile([128, 256], f32, name=f'obB{yt}',
                                  tag=f'obB{yt}', bufs=2)
                    nc.vector.tensor_copy(obB[:], accB[yt])
                    r0, r1 = 128 * yt, 128 * yt + 128
                    nc.sync.dma_start(y_out[b, 0, r0:r1, :], obA[:, 0:256])
                    nc.sync.dma_start(y_out[b, 1, r0:r1, :], obA[:, 256:512])
                    nc.sync.dma_start(y_out[b, 2, r0:r1, :], obB[:])

            stageA(0)
            stageA(1)
            stageB(0)
            for k in range(NSTEP):
                if k + 2 < NSTEP:
                    stageA(k + 2)
                if k + 1 < NSTEP:
                    stageB(k + 1)
                stageC(k)
                if k >= 1:
                    stageD(k - 1)
                    if (k - 1) % NG == NG - 1:
                        drain((k - 1) // NG)
            stageD(NSTEP - 1)
            drain(BLOC - 1)

    _split_multi_waits(nc)
    _PROGRAM = nc
    return nc


def _make_in_maps(brushes: np.ndarray, patches: np.ndarray):
    brushes = np.asarray(brushes, dtype=np.float32)
    patches = np.asarray(patches, dtype=np.float32)
    id4 = np.eye(4, dtype=np.float32)
    # one-hot broadcast matrices: column block g selects strokes 4g..4g+3
    # of a 64-stroke batch half; rows replicated so base partition is 0
    # (batch 0) or 64 (batch 1), matching the S_tab operand slice.
    m = np.arange(128)[:, None] % 64
    g_id = np.arange(NG)[None, None, :]
    p_id = np.arange(128)[None, :, None]
    bm = np.ascontiguousarray(
        (m[:, :, None] == 4 * g_id + p_id // 32)
        .transpose(0, 2, 1).reshape(128, NG * 128).astype(np.float32))
    qio = ((np.arange(128) % 32) - 15.0).astype(np.float32).reshape(128, 1)
    itb = np.broadcast_to(np.arange(IMAGE, dtype=np.float32),
                          (128, IMAGE)).astype(np.float16)
    in_maps = []
    for k in range(NCORES):
        bsl = brushes[BLOC * k: BLOC * (k + 1)]        # [2, 64, 2]
        g_in = np.ascontiguousarray(
            bsl.transpose(0, 2, 1).reshape(4, N))       # rows b0x,b0y,b1x,b1y
        psl = patches[BLOC * k: BLOC * (k + 1)]         # [2, 64, 3, 32, 32]
        pr = psl.reshape(BLOC, NG, 4, C, PH, PW)[..., ::-1, ::-1]
        # -> [b, j, q', g, c, p'] -> [b, 128, NG*C*PH]
        pt = np.ascontiguousarray(pr.transpose(0, 2, 5, 1, 3, 4)).reshape(
            BLOC, 128, NG * C * PH).astype(np.float16)
        in_maps.append({'g_in': g_in, 'pt_in': pt, 'id4': id4, 'bmat': bm,
                        'qio_in': qio, 'it_in': itb})
    return in_maps


def kernel(brushes: np.ndarray, patches: np.ndarray) -> np.ndarray:
    from concourse.bass_utils import run_bass_kernel_spmd

    nc = _build_program()
    in_maps = _make_in_maps(brushes, patches)
    res = run_bass_kernel_spmd(nc, in_maps, list(range(NCORES)))
    out = np.concatenate([res.results[k]['y_out'] for k in range(NCORES)],
                         axis=0)
    return out
